# revision 7
# baseline (speedup 1.0000x reference)
"""Cosine-similarity (pairwise, normalized by sqrt(|a||b|)+eps) Trainium2 kernel.

Problem: first_vector [8192, 512] f32, second_vector [8192, 512] f32,
output sim [8192, 8192] f32 with
    sim = (A @ B.T) / (sqrt(|A_n| * |B_m|) + 1e-6)        (normalize=1)

Strategy (8 NeuronCores, SPMD, no collectives):
  * 2D shard: 4-way over A rows x 2-way over B rows. Core c=(ni,mj)
    computes the [2048, 4096] output slab at (ni*2048, mj*4096).
  * Inputs are staged to DRAM d-major (transposed) in fp16, blocked as
    [n_block, partition, k_chunk, n] so every DMA line is 4 KB
    contiguous. This removes all on-device transposes (the fp16 GEMM
    contracts over d, which must sit on partitions for both operands)
    and cuts input traffic to 6 MB/core.
  * Norms on device, all full-width ops (skinny 1-partition ops are
    ~7 ns/elem -- poison): square the block (round-robin over
    DVE/ACT/GpSimd), reduce over partitions with an all-ones [128,128]
    PE matmul whose output lands already replicated in PSUM, then
    scale = ssq^-1/4 in two wide ACT ops (Dsqrt then Sqrt with
    input-scale 2), and multiply into the fp16 operand per k-chunk.
    out = (s_a A)(s_b B)^T is then already normalized (the reference's
    +eps is dropped; rel. impact ~5e-8).
  * Main GEMM per (row-tile t, column quad): 4 PSUM groups accumulate
    in parallel, k outer / column-block inner so 4 consecutive matmuls
    share the same stationary operand. Evacuation (alternating
    DVE/ACT) casts to fp16 into [128, 2048] staging tiles stored with
    512 KB DMAs (4 KB per partition line). The f32 upcast happens on
    host.
  * Block preps are emitted just-in-time so the PE never waits on prep
    work that isn't needed yet (and never before the producing prep --
    Tile tracks deps by emission order, so a use must follow its def).
"""

import numpy as np

_N, _M, _D = 8192, 8192, 512
_P = 128
_GRID_N, _GRID_M = 4, 2
_AN = _N // _GRID_N        # A rows per core (2048)
_BM = _M // _GRID_M        # B rows per core (4096)
_KC = _D // _P             # contraction chunks (4)
_NB = 512                  # norm/scale block width (one PSUM bank of f32)
_ABL = _AN // _NB          # A blocks (4)
_BBL = _BM // _NB          # B blocks (8)
_QUAD = 4                  # B blocks per output store group

_USE_DSQRT = False         # Dsqrt is in no TRN2 ACT table; keep recip+sqrt+sqrt

TRACE = False              # test harness sets True to collect an NTFF profile
LAST_RESULTS = None        # BassKernelResults of the last run (for test.py)

_NC_CACHE = {}


def _build_nc(normalize: bool):
    import concourse.bass as bass
    import concourse.mybir as mybir
    import concourse.tile as tile
    from concourse import bacc

    f32 = mybir.dt.float32
    f16 = mybir.dt.float16
    MUL = mybir.AluOpType.mult
    SQ = mybir.ActivationFunctionType.Square
    SQRT = mybir.ActivationFunctionType.Sqrt
    nc = bacc.Bacc("TRN2", target_bir_lowering=False, debug=False,
                   enable_asserts=False)

    a_d = nc.declare_dram_parameter("a", [_ABL, _P, _KC, _NB], f16,
                                    isOutput=False)
    b_d = nc.declare_dram_parameter("b", [_BBL, _P, _KC, _NB], f16,
                                    isOutput=False)
    out_d = nc.declare_dram_parameter("out", [_AN, _BM], f16, isOutput=True)

    with tile.TileContext(nc) as tc:
        with (
            tc.tile_pool(name="const", bufs=1) as const_pool,
            tc.tile_pool(name="persist", bufs=1) as persist,
            tc.tile_pool(name="stage", bufs=3) as stage,
            tc.tile_pool(name="sqp", bufs=2) as sqp,
            tc.tile_pool(name="srp", bufs=2) as srp,
            tc.tile_pool(name="spsum", bufs=2, space=bass.MemorySpace.PSUM) as spsum,
            tc.tile_pool(name="mpsum", bufs=5, space=bass.MemorySpace.PSUM) as mpsum,
            tc.tile_pool(name="ostage", bufs=3) as ostage,
        ):
            ones = const_pool.tile([_P, _P], f16)
            nc.vector.memset(ones[:], 1.0)

            # Preload the ACT tables (Dsqrt/Sqrt + Copy) off the
            # critical path.
            warm_in = const_pool.tile([1, 2], f32)
            nc.vector.memset(warm_in[:], 1.0)
            warm_s = const_pool.tile([1, 2], f32)
            nc.scalar.activation(warm_s[:], warm_in[:], SQRT)
            warm_c = const_pool.tile([1, 2], f16)
            nc.scalar.copy(warm_c[:], warm_in[:])

            scaledA = [persist.tile([_P, _KC, _NB], f16, name=f"sA{b}",
                                    tag=f"sA{b}") for b in range(_ABL)]
            scaledB = [persist.tile([_P, _KC, _NB], f16, name=f"sB{s}",
                                    tag=f"sB{s}") for s in range(_BBL)]

            pcount = [0]

            def prep(src_d, bidx, dst):
                """Load block bidx of src_d, compute per-column scale
                ssq^-1/4 and write the scaled fp16 block into dst."""
                if not normalize:
                    nc.sync.dma_start(dst[:], src_d[bidx])
                    return
                raw = stage.tile([_P, _KC, _NB], f16, tag="raw")
                nc.sync.dma_start(raw[:], src_d[bidx])
                sq = sqp.tile([_P, _KC, _NB], f16, tag="sq")
                e = pcount[0] % 3
                pcount[0] += 1
                if e == 0:
                    nc.vector.tensor_tensor(sq[:], raw[:], raw[:], MUL)
                elif e == 1:
                    nc.scalar.activation(sq[:], raw[:], SQ)
                else:
                    nc.gpsimd.tensor_tensor(sq[:], raw[:], raw[:], MUL)
                # Partition-reduce with an all-ones stationary operand:
                # every output partition gets the column ssq.
                ssq = spsum.tile([_P, _NB], f32)
                for k in range(_KC):
                    nc.tensor.matmul(ssq[:], lhsT=ones[:], rhs=sq[:, k],
                                     start=(k == 0), stop=(k == _KC - 1))
                srep = srp.tile([_P, _NB], f16, tag="srep")
                rec = srp.tile([_P, _NB], f32, tag="rec")
                nc.vector.reciprocal(rec[:], ssq[:])
                half = srp.tile([_P, _NB], f32, tag="half")
                nc.scalar.activation(half[:], rec[:], SQRT)
                nc.scalar.activation(srep[:], half[:], SQRT)
                for k in range(_KC):
                    eng = nc.vector if (pcount[0] + k) % 2 else nc.gpsimd
                    eng.tensor_tensor(dst[:, k], raw[:, k], srep[:], MUL)

            prep(b_d, 0, scaledB[0])
            prep(b_d, 1, scaledB[1])
            prep(b_d, 2, scaledB[2])
            prep(b_d, 3, scaledB[3])
            prep(a_d, 0, scaledA[0])

            cidx = 0
            for q in range(_BM // (_QUAD * _NB)):
                for t in range(_AN // _P):
                    if q == 0:
                        if t == 1:
                            prep(a_d, 1, scaledA[1])
                        elif t == 3:
                            prep(a_d, 2, scaledA[2])
                        elif t == 5:
                            prep(a_d, 3, scaledA[3])
                        elif t == 8:
                            prep(b_d, 4, scaledB[4])
                        elif t == 10:
                            prep(b_d, 5, scaledB[5])
                        elif t == 12:
                            prep(b_d, 6, scaledB[6])
                        elif t == 14:
                            prep(b_d, 7, scaledB[7])
                    ab, ar = divmod(t, _NB // _P)
                    ost = ostage.tile([_P, _QUAD * _NB], f16)
                    pss = [mpsum.tile([_P, _NB], f32, name="ps", tag="ps")
                           for h in range(_QUAD)]
                    for k in range(_KC):
                        lhsT = scaledA[ab][:, k, ar * _P:(ar + 1) * _P]
                        for h in range(_QUAD):
                            nc.tensor.matmul(
                                pss[h],
                                lhsT=lhsT,
                                rhs=scaledB[q * _QUAD + h][:, k, :],
                                start=(k == 0),
                                stop=(k == _KC - 1),
                            )
                    for h in range(_QUAD):
                        dst = ost[:, h * _NB:(h + 1) * _NB]
                        if cidx % 2 == 0:
                            nc.vector.tensor_copy(dst, pss[h])
                        else:
                            nc.scalar.copy(dst, pss[h])
                        cidx += 1
                    nc.sync.dma_start(
                        out_d[t * _P:(t + 1) * _P,
                              q * _QUAD * _NB:(q + 1) * _QUAD * _NB],
                        ost[:],
                    )

    nc.compile()
    return nc


def _get_nc(normalize: bool):
    key = bool(normalize)
    if key not in _NC_CACHE:
        _NC_CACHE[key] = _build_nc(key)
    return _NC_CACHE[key]


def _dmajor_blocks(slab16: np.ndarray, nblocks: int) -> np.ndarray:
    """[rows, 512] fp16 -> [nblocks, 128, 4, 512] with element
    [bl, p, k, n] = slab[bl*512 + n, k*128 + p] (d-major, 4 KB lines)."""
    t = slab16.T.reshape(_KC, _P, nblocks, _NB).transpose(2, 1, 0, 3)
    return np.ascontiguousarray(t)


def kernel(first_vector, second_vector, normalize):
    global LAST_RESULTS
    from concourse.bass_utils import run_bass_kernel_spmd

    a = np.asarray(first_vector, dtype=np.float32)
    b = np.asarray(second_vector, dtype=np.float32)
    assert a.shape == (_N, _D) and b.shape == (_M, _D)
    norm = bool(int(np.asarray(normalize)))

    a16 = a.astype(np.float16)
    b16 = b.astype(np.float16)

    nc = _get_nc(norm)

    in_maps = []
    for c in range(_GRID_N * _GRID_M):
        ni, mj = divmod(c, _GRID_M)
        in_maps.append(
            {
                "a": _dmajor_blocks(a16[ni * _AN:(ni + 1) * _AN], _ABL),
                "b": _dmajor_blocks(b16[mj * _BM:(mj + 1) * _BM], _BBL),
            }
        )

    res = run_bass_kernel_spmd(
        nc, in_maps, core_ids=list(range(_GRID_N * _GRID_M)), trace=TRACE
    )
    LAST_RESULTS = res

    out16 = np.empty((_N, _M), dtype=np.float16)
    for c in range(_GRID_N * _GRID_M):
        ni, mj = divmod(c, _GRID_M)
        out16[ni * _AN:(ni + 1) * _AN, mj * _BM:(mj + 1) * _BM] = (
            res.results[c]["out"]
        )
    return out16.astype(np.float32)
